# revision 17
# baseline (speedup 1.0000x reference)
"""Causal single-head attention (B=4, S=2048, D=1024, f32) on 8 trn2 cores.

Sharding: data-parallel over batch (4) x 2-way split per batch, with the
K/V projections split by sequence parity and exchanged between the pair
via a 2-core AllGather (no duplicated projection work).

Core c handles batch b=c//2 and parity h=c%2: its query rows (and its
share of the K/V projection rows) are the 128-row blocks {2j+h : j=0..7}.
Each core projects Q/K/V only for its own 1024 rows, the pair exchanges
the K/V halves through DRAM bounce buffers + AllGather (replica groups
[[0,1],[2,3],[4,5],[6,7]]), and every core then runs block-causal
attention for its queries against the full 2048-row K/V.

K/V live in SBUF in *global* parity-block order (parity-0 blocks
ascending in the first half, parity-1 in the second) — exactly the
AllGather output order — so the instruction stream is identical on all
8 cores.  The residual per-core causal differences (whether the
boundary block of each parity is diagonal / fully-valid / fully-masked)
are carried by two 128x128 multiplicative mask inputs mA/mB.

All matmuls run in bf16 (inputs pre-cast/pre-transposed on the host),
f32 PSUM accumulation, f32 softmax normalization and f32 output.
Scores are computed transposed (s^T[k,q]) per key block with the widest
legal free dim, exp(scale*s) goes psum->sbuf on the scalar engine with
no max pass (|scale*s| <= ~5 for this data), p^T is the stationary
operand of the context matmul, and row sums come from ones^T @ p^T
matmuls accumulated in PSUM.

A tiny warmup AllGather issues at kernel start so the NRT collective
channel setup (~40us) overlaps with the projection compute instead of
delaying the K exchange.
"""

import numpy as np
import ml_dtypes

B, S, D = 4, 2048, 1024
P = 128
DI = D // P          # 8 contraction subtiles
NBLK = S // P        # 16 sequence blocks
NSLOT = 8            # query blocks per core
QCORE = NSLOT * P    # 1024 query rows per core
HALF = S // 2        # 1024 own-parity rows per core
SCALE = 1.0 / 32.0   # 1/sqrt(D)
BF16 = ml_dtypes.bfloat16
GROUPS = [[0, 1], [2, 3], [4, 5], [6, 7]]

_PROGRAM = None


def _build_program():
    import concourse.bacc as bacc
    import concourse.mybir as mybir
    import concourse.tile as tile

    dt = mybir.dt
    f32 = dt.float32
    bf = dt.bfloat16
    Exp = mybir.ActivationFunctionType.Exp

    nc = bacc.Bacc("TRN2")
    xT = nc.dram_tensor("xT", [D, HALF], bf, kind="ExternalInput")
    wqT = nc.dram_tensor("wqT", [D, D], bf, kind="ExternalInput")
    wkT = nc.dram_tensor("wkT", [D, D], bf, kind="ExternalInput")
    wvT = nc.dram_tensor("wvT", [D, D], bf, kind="ExternalInput")
    mA = nc.dram_tensor("mA", [P, P], bf, kind="ExternalInput")
    mB = nc.dram_tensor("mB", [P, P], bf, kind="ExternalInput")
    y = nc.dram_tensor("y", [QCORE, D], f32, kind="ExternalOutput")

    with (
        tile.TileContext(nc) as tc,
        tc.tile_pool(name="pers", bufs=1) as pers,
        tc.tile_pool(name="dram", bufs=1, space="DRAM") as dram,
    ):
        mA_sb = pers.tile([P, P], bf, tag="mA", name="mA")
        nc.gpsimd.dma_start(mA_sb[:], mA[:])
        mB_sb = pers.tile([P, P], bf, tag="mB", name="mB")
        nc.gpsimd.dma_start(mB_sb[:], mB[:])
        ones_sb = pers.tile([P, 1], bf, tag="ones", name="ones")
        nc.gpsimd.memset(ones_sb[:], 1.0)

        # warmup collective: pays the NRT channel-setup latency up front
        warm_in = dram.tile([P, 1], bf, tag="warm_in", name="warm_in")
        warm_out = dram.tile([2 * P, 1], bf, tag="warm_out", name="warm_out")
        nc.gpsimd.dma_start(warm_in[:], ones_sb[:])
        nc.gpsimd.collective_compute(
            "AllGather",
            mybir.AluOpType.bypass,
            replica_groups=GROUPS,
            ins=[warm_in.opt()],
            outs=[warm_out.opt()],
        )

        # persistent per-core tensors (bf16), global parity-block layout:
        # kT [o, s_local], qT [o, q], v [s_local, o]
        kT_o = [pers.tile([P, S], bf, tag=f"kT{oi}", name=f"kT{oi}") for oi in range(DI)]
        qT_o = [pers.tile([P, QCORE], bf, tag=f"qT{oi}", name=f"qT{oi}") for oi in range(DI)]
        v_s = [pers.tile([P, D], bf, tag=f"v{si}", name=f"v{si}") for si in range(NBLK)]

        # bounce buffers for the K/V halves exchange (Shared outputs are
        # only supported for >4-core groups, so these stay Local)
        kb_in = dram.tile([HALF, HALF], bf, tag="kb_in", name="kb_in")
        vb_in = dram.tile([HALF, D], bf, tag="vb_in", name="vb_in")
        kb_out = dram.tile([S, HALF], bf, tag="kb_out", name="kb_out")
        vb_out = dram.tile([S, D], bf, tag="vb_out", name="vb_out")

        # ---- load + projections (own 1024 rows only) ----
        NSC = HALF // 512  # 2 s-chunks
        with (
            tc.tile_pool(name="ld", bufs=1) as ld,
            tc.tile_pool(name="ppsum", bufs=4, space="PSUM") as ppsum,
        ):
            xT3 = xT.rearrange("(di p) s -> di p s", p=P)
            wq3 = wqT.rearrange("(di p) o -> di p o", p=P)
            wk3 = wkT.rearrange("(di p) o -> di p o", p=P)
            wv3 = wvT.rearrange("(di p) o -> di p o", p=P)
            # One max-size (256KB) DMA per [128, 1024] tile, spread across
            # the three DMA-capable queues so the ring-credit limit (4
            # outstanding per ring) doesn't serialize the 8MB input load:
            # wk then wq on sync, xT on scalar, wv on gpsimd (done well
            # before the collective staging needs that queue).
            wk_d, xT_t, wv_d, wq_d = [], [], [], []
            for di in range(DI):
                t = ld.tile([P, D], bf, tag=f"wk{di}", name=f"wk{di}")
                nc.sync.dma_start(t[:], wk3[di])
                wk_d.append(t)
            for di in range(DI):
                t = ld.tile([P, HALF], bf, tag=f"xT{di}", name=f"xT{di}")
                nc.scalar.dma_start(t[:], xT3[di])
                xT_t.append(t)
            for di in range(DI):
                t = ld.tile([P, D], bf, tag=f"wv{di}", name=f"wv{di}")
                nc.gpsimd.dma_start(t[:], wv3[di])
                wv_d.append(t)
            for di in range(DI):
                t = ld.tile([P, D], bf, tag=f"wq{di}", name=f"wq{di}")
                nc.sync.dma_start(t[:], wq3[di])
                wq_d.append(t)

            def proj(lhs_fn, rhs_fn, out_tile, out_cols, n):
                ps = ppsum.tile([P, n], f32, tag="ppsum", name="ppsum")
                for di in range(DI):
                    nc.tensor.matmul(
                        ps[:],
                        lhs_fn(di),
                        rhs_fn(di),
                        start=(di == 0),
                        stop=(di == DI - 1),
                    )
                nc.vector.tensor_copy(out_tile[:, out_cols], ps[:])

            with tc.tile_pool(name="kst", bufs=1) as kst:
                kstage = [
                    kst.tile([P, HALF], bf, tag=f"ks{oi}", name=f"ks{oi}")
                    for oi in range(DI)
                ]
                for sc in range(NSC):  # kT own half: [o, s_own]
                    cols = slice(sc * 512, (sc + 1) * 512)
                    for oi in range(DI):
                        oc = slice(oi * P, (oi + 1) * P)
                        proj(
                            lambda di, oc=oc: wk_d[di][:, oc],
                            lambda di, cols=cols: xT_t[di][:, cols],
                            kstage[oi], cols, 512,
                        )
                for oi in range(DI):  # stage out + exchange K
                    nc.gpsimd.dma_start(
                        kb_in[oi * P : (oi + 1) * P, :], kstage[oi][:]
                    )
            nc.gpsimd.collective_compute(
                "AllGather",
                mybir.AluOpType.bypass,
                replica_groups=GROUPS,
                ins=[kb_in.opt()],
                outs=[kb_out.opt()],
            )

            with tc.tile_pool(name="vst", bufs=1) as vst:
                vstage = [
                    vst.tile([P, D], bf, tag=f"vs{si}", name=f"vs{si}")
                    for si in range(NSLOT)
                ]
                for si in range(NSLOT):  # v own half: [s_own, o]
                    lo = si * P
                    for oh in range(D // 512):
                        cols = slice(oh * 512, (oh + 1) * 512)
                        proj(
                            lambda di, lo=lo: xT_t[di][:, lo : lo + P],
                            lambda di, cols=cols: wv_d[di][:, cols],
                            vstage[si], cols, 512,
                        )
                    nc.gpsimd.dma_start(
                        vb_in[si * P : (si + 1) * P, :], vstage[si][:]
                    )
            nc.gpsimd.collective_compute(
                "AllGather",
                mybir.AluOpType.bypass,
                replica_groups=GROUPS,
                ins=[vb_in.opt()],
                outs=[vb_out.opt()],
            )

            for qc in range(QCORE // 512):  # qT: [o, q] own rows
                cols = slice(qc * 512, (qc + 1) * 512)
                for oi in range(DI):
                    oc = slice(oi * P, (oi + 1) * P)
                    proj(
                        lambda di, oc=oc: wq_d[di][:, oc],
                        lambda di, cols=cols: xT_t[di][:, cols],
                        qT_o[oi], cols, 512,
                    )

            # load back the gathered K (scalar; idle until exp) / V (sync)
            for oi in range(DI):
                nc.scalar.dma_start(
                    kT_o[oi][:, 0:HALF], kb_out[oi * P : (oi + 1) * P, :]
                )
                nc.scalar.dma_start(
                    kT_o[oi][:, HALF:S],
                    kb_out[HALF + oi * P : HALF + (oi + 1) * P, :],
                )
            for si in range(NSLOT):
                nc.sync.dma_start(
                    v_s[si][:], vb_out[si * P : (si + 1) * P, :]
                )
                nc.sync.dma_start(
                    v_s[NSLOT + si][:], vb_out[HALF + si * P : HALF + (si + 1) * P, :]
                )

        # ---- attention ----
        # local key block kb: 0..7 -> global 2*kb (parity 0),
        #                     8..15 -> global 2*(kb-8)+1 (parity 1).
        # key block kb is attended by query slots j >= kb%8.
        pT = [pers.tile([P, QCORE], bf, tag=f"pT{kb}", name=f"pT{kb}") for kb in range(NBLK)]
        with (
            tc.tile_pool(name="spsum", bufs=2, space="PSUM") as spsum,
            tc.tile_pool(name="rpsum", bufs=2, space="PSUM") as rpsum,
            tc.tile_pool(name="cpsum", bufs=4, space="PSUM") as cpsum,
            tc.tile_pool(name="small", bufs=2) as small,
            tc.tile_pool(name="reciprocals", bufs=1) as rpool,
        ):
            # scores s^T[k, q] per key block, widest legal free dim
            for kb in range(NBLK):
                ksl = slice(kb * P, (kb + 1) * P)
                q0 = (kb % NSLOT) * P
                while q0 < QCORE:
                    w = min(512, QCORE - q0)
                    ps = spsum.tile([P, 512], f32, tag="spsum", name="spsum")
                    for oi in range(DI):
                        nc.tensor.matmul(
                            ps[:, 0:w],
                            kT_o[oi][:, ksl],
                            qT_o[oi][:, q0 : q0 + w],
                            start=(oi == 0),
                            stop=(oi == DI - 1),
                        )
                    nc.scalar.activation(
                        pT[kb][:, q0 : q0 + w], ps[:, 0:w], Exp, scale=SCALE
                    )
                    q0 += w
            # boundary masks (multiplicative, post-exp)
            for j in range(NSLOT):
                qc = slice(j * P, (j + 1) * P)
                nc.vector.tensor_mul(pT[j][:, qc], pT[j][:, qc], mA_sb[:])
                nc.vector.tensor_mul(
                    pT[NSLOT + j][:, qc], pT[NSLOT + j][:, qc], mB_sb[:]
                )
            # row sums (pT.T @ ones -> [128,1] psum, q on partitions)
            # and context: ctx[q, o] = sum_k p^T[k,q] * v[k,o]
            for j in reversed(range(NSLOT)):  # big slot first, small tail last
                qc = slice(j * P, (j + 1) * P)
                kbs = list(range(0, j + 1)) + list(range(NSLOT, NSLOT + j + 1))
                rsp = rpsum.tile([P, 1], f32, tag="rsp", name="rsp")
                for i, kb in enumerate(kbs):
                    nc.tensor.matmul(
                        rsp[:],
                        pT[kb][:, qc],
                        ones_sb[:, 0:1],
                        start=(i == 0),
                        stop=(i == len(kbs) - 1),
                    )
                recip = rpool.tile([P, 1], f32, tag=f"recip{j}", name=f"recip{j}")
                nc.vector.reciprocal(recip[:], rsp[:])
                for oh in range(D // 512):
                    ocols = slice(oh * 512, (oh + 1) * 512)
                    cps = cpsum.tile([P, 512], f32, tag="cpsum", name="cpsum")
                    for i, kb in enumerate(kbs):
                        nc.tensor.matmul(
                            cps[:],
                            pT[kb][:, qc],
                            v_s[kb][:, ocols],
                            start=(i == 0),
                            stop=(i == len(kbs) - 1),
                        )
                    ct = small.tile([P, 512], f32, tag="ct", name="ct")
                    nc.vector.tensor_scalar_mul(ct[:], cps[:], recip[:, 0:1])
                    nc.sync.dma_start(y[qc, ocols], ct[:])
    nc.finalize()
    return nc


def _get_program():
    global _PROGRAM
    if _PROGRAM is None:
        _PROGRAM = _build_program()
    return _PROGRAM


def _host_prep(x, Wq, Wk, Wv):
    """Per-core input maps: transposed/cast weights, own-parity x^T, masks."""
    x = np.asarray(x, dtype=np.float32)
    tri_np = (np.arange(P)[None, :] >= np.arange(P)[:, None]).astype(BF16)
    ones_np = np.ones((P, P), dtype=BF16)
    zeros_np = np.zeros((P, P), dtype=BF16)
    wqT = np.ascontiguousarray(np.asarray(Wq, dtype=np.float32).T).astype(BF16)
    wkT = np.ascontiguousarray(np.asarray(Wk, dtype=np.float32).T).astype(BF16)
    wvT = np.ascontiguousarray(np.asarray(Wv, dtype=np.float32).T).astype(BF16)
    in_maps = []
    for c in range(8):
        b, h = c // 2, c % 2
        own = [2 * j + h for j in range(NSLOT)]
        xTb = np.asarray(x[b]).T.reshape(D, NBLK, P)[:, own, :].reshape(D, HALF)
        in_maps.append(
            {
                "xT": np.ascontiguousarray(xTb).astype(BF16),
                "wqT": wqT,
                "wkT": wkT,
                "wvT": wvT,
                "mA": tri_np if h == 0 else ones_np,
                "mB": zeros_np if h == 0 else tri_np,
            }
        )
    return in_maps


def run(x, Wq, Wk, Wv, **spmd_kwargs):
    """Run on all 8 cores; returns (out [B,S,D] f32, BassKernelResults)."""
    from concourse.bass_utils import run_bass_kernel_spmd

    nc = _get_program()
    in_maps = _host_prep(x, Wq, Wk, Wv)
    res = run_bass_kernel_spmd(nc, in_maps, core_ids=list(range(8)), **spmd_kwargs)
    out = np.empty((B, S, D), dtype=np.float32)
    for c in range(8):
        b, h = c // 2, c % 2
        yc = res.results[c]["y"]
        for j in range(NSLOT):
            g = 2 * j + h
            out[b, g * P : (g + 1) * P, :] = yc[j * P : (j + 1) * P, :]
    return out, res


def kernel(x, Wq, Wk, Wv):
    out, _ = run(x, Wq, Wk, Wv)
    return out


# revision 19
# speedup vs baseline: 1.1268x; 1.1268x over previous
"""Causal single-head attention (B=4, S=2048, D=1024, f32) on 8 trn2 cores.

Sharding: data-parallel over batch (4) x 2-way split per batch, with the
K/V projections split by sequence parity and exchanged between the pair
via a 2-core AllGather (no duplicated projection work).

Core c handles batch b=c//2 and parity h=c%2: its query rows (and its
share of the K/V projection rows) are the 128-row blocks {2j+h : j=0..7}.
Each core projects Q/K/V only for its own 1024 rows, the pair exchanges
the K/V halves through DRAM bounce buffers + AllGather (replica groups
[[0,1],[2,3],[4,5],[6,7]]), and every core then runs block-causal
attention for its queries against the full 2048-row K/V.

K/V live in SBUF in *global* parity-block order (parity-0 blocks
ascending in the first half, parity-1 in the second) — exactly the
AllGather output order — so the instruction stream is identical on all
8 cores.  The residual per-core causal differences (whether the
boundary block of each parity is diagonal / fully-valid / fully-masked)
are carried by two 128x128 multiplicative mask inputs mA/mB.

All matmuls run in bf16 (inputs pre-cast/pre-transposed on the host),
f32 PSUM accumulation, f32 softmax normalization and f32 output.
Scores are computed transposed (s^T[k,q]) per key block with the widest
legal free dim, exp(scale*s) goes psum->sbuf on the scalar engine with
no max pass (|scale*s| <= ~5 for this data), p^T is the stationary
operand of the context matmul, and row sums come from ones^T @ p^T
matmuls accumulated in PSUM.

A tiny warmup AllGather issues at kernel start so the NRT collective
channel setup (~40us) overlaps with the projection compute instead of
delaying the K exchange.
"""

import numpy as np
import ml_dtypes

B, S, D = 4, 2048, 1024
P = 128
DI = D // P          # 8 contraction subtiles
NBLK = S // P        # 16 sequence blocks
NSLOT = 8            # query blocks per core
QCORE = NSLOT * P    # 1024 query rows per core
HALF = S // 2        # 1024 own-parity rows per core
SCALE = 1.0 / 32.0   # 1/sqrt(D)
BF16 = ml_dtypes.bfloat16
GROUPS = [[0, 1], [2, 3], [4, 5], [6, 7]]

_PROGRAM = None


def _build_program():
    import concourse.bacc as bacc
    import concourse.mybir as mybir
    import concourse.tile as tile

    dt = mybir.dt
    f32 = dt.float32
    bf = dt.bfloat16
    Exp = mybir.ActivationFunctionType.Exp

    nc = bacc.Bacc("TRN2")
    xT = nc.dram_tensor("xT", [D, HALF], bf, kind="ExternalInput")
    wqT = nc.dram_tensor("wqT", [D, D], bf, kind="ExternalInput")
    wkT = nc.dram_tensor("wkT", [D, D], bf, kind="ExternalInput")
    wvT = nc.dram_tensor("wvT", [D, D], bf, kind="ExternalInput")
    mA = nc.dram_tensor("mA", [P, P], bf, kind="ExternalInput")
    mB = nc.dram_tensor("mB", [P, P], bf, kind="ExternalInput")
    y = nc.dram_tensor("y", [QCORE, D], f32, kind="ExternalOutput")

    with (
        tile.TileContext(nc) as tc,
        tc.tile_pool(name="pers", bufs=1) as pers,
        tc.tile_pool(name="dram", bufs=1, space="DRAM") as dram,
    ):
        # warmup collective: pays the NRT channel-setup latency up front.
        # Its input is a tiny dram->dram copy issued before the big input
        # loads so its completion (and the CC doorbell) isn't queued behind
        # 8MB of weight traffic.
        warm_in = dram.tile([1, P], bf, tag="warm_in", name="warm_in")
        warm_out = dram.tile([2, P], bf, tag="warm_out", name="warm_out")
        nc.gpsimd.dma_start(warm_in[:], mA[0:1, :])
        nc.gpsimd.collective_compute(
            "AllGather",
            mybir.AluOpType.bypass,
            replica_groups=GROUPS,
            ins=[warm_in.opt()],
            outs=[warm_out.opt()],
        )

        mA_sb = pers.tile([P, P], bf, tag="mA", name="mA")
        nc.gpsimd.dma_start(mA_sb[:], mA[:])
        mB_sb = pers.tile([P, P], bf, tag="mB", name="mB")
        nc.gpsimd.dma_start(mB_sb[:], mB[:])
        ones_sb = pers.tile([P, 1], bf, tag="ones", name="ones")
        nc.gpsimd.memset(ones_sb[:], 1.0)

        # persistent per-core tensors (bf16), global parity-block layout:
        # kT [o, s_local], qT [o, q], v [s_local, o]
        kT_o = [pers.tile([P, S], bf, tag=f"kT{oi}", name=f"kT{oi}") for oi in range(DI)]
        qT_o = [pers.tile([P, QCORE], bf, tag=f"qT{oi}", name=f"qT{oi}") for oi in range(DI)]
        v_s = [pers.tile([P, D], bf, tag=f"v{si}", name=f"v{si}") for si in range(NBLK)]

        # bounce buffers for the K/V halves exchange (Shared outputs are
        # only supported for >4-core groups, so these stay Local)
        kb_in = dram.tile([HALF, HALF], bf, tag="kb_in", name="kb_in")
        vb_in = dram.tile([HALF, D], bf, tag="vb_in", name="vb_in")
        kb_out = dram.tile([S, HALF], bf, tag="kb_out", name="kb_out")
        vb_out = dram.tile([S, D], bf, tag="vb_out", name="vb_out")

        # ---- load + projections (own 1024 rows only) ----
        NSC = HALF // 512  # 2 s-chunks
        with (
            tc.tile_pool(name="ld", bufs=1) as ld,
            tc.tile_pool(name="ppsum", bufs=4, space="PSUM") as ppsum,
        ):
            xT3 = xT.rearrange("(di p) s -> di p s", p=P)
            wq3 = wqT.rearrange("(di p) o -> di p o", p=P)
            wk3 = wkT.rearrange("(di p) o -> di p o", p=P)
            wv3 = wvT.rearrange("(di p) o -> di p o", p=P)
            # Inputs spread across the three DMA-capable queues, ordered by
            # first use so the first projection groups' operands land first:
            # sync: wk then wq; scalar: xT first halves then second halves;
            # gpsimd: wv (after the tiny warmup-CC traffic).
            wk_d, xT_t, wv_d, wq_d = [], [], [], []
            for di in range(DI):
                t = ld.tile([P, D], bf, tag=f"wk{di}", name=f"wk{di}")
                nc.sync.dma_start(t[:], wk3[di])
                wk_d.append(t)
            for di in range(DI):
                t = ld.tile([P, HALF], bf, tag=f"xT{di}", name=f"xT{di}")
                nc.scalar.dma_start(t[:, 0:512], xT3[di, :, 0:512])
                xT_t.append(t)
            for di in range(DI):
                t = ld.tile([P, D], bf, tag=f"wv{di}", name=f"wv{di}")
                nc.gpsimd.dma_start(t[:], wv3[di])
                wv_d.append(t)
            for di in range(DI):
                nc.scalar.dma_start(xT_t[di][:, 512:HALF], xT3[di, :, 512:HALF])
            for di in range(DI):
                t = ld.tile([P, D], bf, tag=f"wq{di}", name=f"wq{di}")
                nc.sync.dma_start(t[:], wq3[di])
                wq_d.append(t)

            def proj(lhs_fn, rhs_fn, out_tile, out_cols, n):
                ps = ppsum.tile([P, n], f32, tag="ppsum", name="ppsum")
                for di in range(DI):
                    nc.tensor.matmul(
                        ps[:],
                        lhs_fn(di),
                        rhs_fn(di),
                        start=(di == 0),
                        stop=(di == DI - 1),
                    )
                nc.vector.tensor_copy(out_tile[:, out_cols], ps[:])

            with tc.tile_pool(name="kst", bufs=1) as kst:
                kstage = [
                    kst.tile([P, HALF], bf, tag=f"ks{oi}", name=f"ks{oi}")
                    for oi in range(DI)
                ]
                for sc in range(NSC):  # kT own half: [o, s_own]
                    cols = slice(sc * 512, (sc + 1) * 512)
                    for oi in range(DI):
                        oc = slice(oi * P, (oi + 1) * P)
                        proj(
                            lambda di, oc=oc: wk_d[di][:, oc],
                            lambda di, cols=cols: xT_t[di][:, cols],
                            kstage[oi], cols, 512,
                        )
                for oi in range(DI):  # stage out + exchange K
                    nc.gpsimd.dma_start(
                        kb_in[oi * P : (oi + 1) * P, :], kstage[oi][:]
                    )
            nc.gpsimd.collective_compute(
                "AllGather",
                mybir.AluOpType.bypass,
                replica_groups=GROUPS,
                ins=[kb_in.opt()],
                outs=[kb_out.opt()],
            )

            with tc.tile_pool(name="vst", bufs=1) as vst:
                vstage = [
                    vst.tile([P, D], bf, tag=f"vs{si}", name=f"vs{si}")
                    for si in range(NSLOT)
                ]
                for si in range(NSLOT):  # v own half: [s_own, o]
                    lo = si * P
                    for oh in range(D // 512):
                        cols = slice(oh * 512, (oh + 1) * 512)
                        proj(
                            lambda di, lo=lo: xT_t[di][:, lo : lo + P],
                            lambda di, cols=cols: wv_d[di][:, cols],
                            vstage[si], cols, 512,
                        )
                    nc.gpsimd.dma_start(
                        vb_in[si * P : (si + 1) * P, :], vstage[si][:]
                    )
            nc.gpsimd.collective_compute(
                "AllGather",
                mybir.AluOpType.bypass,
                replica_groups=GROUPS,
                ins=[vb_in.opt()],
                outs=[vb_out.opt()],
            )

            for qc in range(QCORE // 512):  # qT: [o, q] own rows
                cols = slice(qc * 512, (qc + 1) * 512)
                for oi in range(DI):
                    oc = slice(oi * P, (oi + 1) * P)
                    proj(
                        lambda di, oc=oc: wq_d[di][:, oc],
                        lambda di, cols=cols: xT_t[di][:, cols],
                        qT_o[oi], cols, 512,
                    )

            # load back the gathered K (scalar; idle until exp) / V (sync)
            for oi in range(DI):
                nc.scalar.dma_start(
                    kT_o[oi][:, 0:HALF], kb_out[oi * P : (oi + 1) * P, :]
                )
                nc.scalar.dma_start(
                    kT_o[oi][:, HALF:S],
                    kb_out[HALF + oi * P : HALF + (oi + 1) * P, :],
                )
            for si in range(NSLOT):
                nc.sync.dma_start(
                    v_s[si][:], vb_out[si * P : (si + 1) * P, :]
                )
                nc.sync.dma_start(
                    v_s[NSLOT + si][:], vb_out[HALF + si * P : HALF + (si + 1) * P, :]
                )

        # ---- attention ----
        # local key block kb: 0..7 -> global 2*kb (parity 0),
        #                     8..15 -> global 2*(kb-8)+1 (parity 1).
        # key block kb is attended by query slots j >= kb%8.
        pT = [pers.tile([P, QCORE], bf, tag=f"pT{kb}", name=f"pT{kb}") for kb in range(NBLK)]
        with (
            tc.tile_pool(name="spsum", bufs=2, space="PSUM") as spsum,
            tc.tile_pool(name="rpsum", bufs=2, space="PSUM") as rpsum,
            tc.tile_pool(name="cpsum", bufs=4, space="PSUM") as cpsum,
            tc.tile_pool(name="small", bufs=2) as small,
            tc.tile_pool(name="reciprocals", bufs=1) as rpool,
        ):
            # scores s^T[k, q] per key block, widest legal free dim
            for kb in range(NBLK):
                ksl = slice(kb * P, (kb + 1) * P)
                q0 = (kb % NSLOT) * P
                while q0 < QCORE:
                    w = min(512, QCORE - q0)
                    ps = spsum.tile([P, 512], f32, tag="spsum", name="spsum")
                    for oi in range(DI):
                        nc.tensor.matmul(
                            ps[:, 0:w],
                            kT_o[oi][:, ksl],
                            qT_o[oi][:, q0 : q0 + w],
                            start=(oi == 0),
                            stop=(oi == DI - 1),
                        )
                    nc.scalar.activation(
                        pT[kb][:, q0 : q0 + w], ps[:, 0:w], Exp, scale=SCALE
                    )
                    q0 += w
            # boundary masks (multiplicative, post-exp)
            for j in range(NSLOT):
                qc = slice(j * P, (j + 1) * P)
                nc.vector.tensor_mul(pT[j][:, qc], pT[j][:, qc], mA_sb[:])
                nc.vector.tensor_mul(
                    pT[NSLOT + j][:, qc], pT[NSLOT + j][:, qc], mB_sb[:]
                )
            # row sums (pT.T @ ones -> [128,1] psum, q on partitions)
            # and context: ctx[q, o] = sum_k p^T[k,q] * v[k,o]
            for j in reversed(range(NSLOT)):  # big slot first, small tail last
                qc = slice(j * P, (j + 1) * P)
                kbs = list(range(0, j + 1)) + list(range(NSLOT, NSLOT + j + 1))
                rsp = rpsum.tile([P, 1], f32, tag="rsp", name="rsp")
                for i, kb in enumerate(kbs):
                    nc.tensor.matmul(
                        rsp[:],
                        pT[kb][:, qc],
                        ones_sb[:, 0:1],
                        start=(i == 0),
                        stop=(i == len(kbs) - 1),
                    )
                recip = rpool.tile([P, 1], f32, tag=f"recip{j}", name=f"recip{j}")
                nc.vector.reciprocal(recip[:], rsp[:])
                for oh in range(D // 512):
                    ocols = slice(oh * 512, (oh + 1) * 512)
                    cps = cpsum.tile([P, 512], f32, tag="cpsum", name="cpsum")
                    for i, kb in enumerate(kbs):
                        nc.tensor.matmul(
                            cps[:],
                            pT[kb][:, qc],
                            v_s[kb][:, ocols],
                            start=(i == 0),
                            stop=(i == len(kbs) - 1),
                        )
                    ct = small.tile([P, 512], f32, tag="ct", name="ct")
                    nc.vector.tensor_scalar_mul(ct[:], cps[:], recip[:, 0:1])
                    nc.sync.dma_start(y[qc, ocols], ct[:])
    nc.finalize()
    return nc


def _get_program():
    global _PROGRAM
    if _PROGRAM is None:
        _PROGRAM = _build_program()
    return _PROGRAM


def _host_prep(x, Wq, Wk, Wv):
    """Per-core input maps: transposed/cast weights, own-parity x^T, masks."""
    x = np.asarray(x, dtype=np.float32)
    tri_np = (np.arange(P)[None, :] >= np.arange(P)[:, None]).astype(BF16)
    ones_np = np.ones((P, P), dtype=BF16)
    zeros_np = np.zeros((P, P), dtype=BF16)
    wqT = np.ascontiguousarray(np.asarray(Wq, dtype=np.float32).T).astype(BF16)
    wkT = np.ascontiguousarray(np.asarray(Wk, dtype=np.float32).T).astype(BF16)
    wvT = np.ascontiguousarray(np.asarray(Wv, dtype=np.float32).T).astype(BF16)
    in_maps = []
    for c in range(8):
        b, h = c // 2, c % 2
        own = [2 * j + h for j in range(NSLOT)]
        xTb = np.asarray(x[b]).T.reshape(D, NBLK, P)[:, own, :].reshape(D, HALF)
        in_maps.append(
            {
                "xT": np.ascontiguousarray(xTb).astype(BF16),
                "wqT": wqT,
                "wkT": wkT,
                "wvT": wvT,
                "mA": tri_np if h == 0 else ones_np,
                "mB": zeros_np if h == 0 else tri_np,
            }
        )
    return in_maps


def run(x, Wq, Wk, Wv, **spmd_kwargs):
    """Run on all 8 cores; returns (out [B,S,D] f32, BassKernelResults)."""
    from concourse.bass_utils import run_bass_kernel_spmd

    nc = _get_program()
    in_maps = _host_prep(x, Wq, Wk, Wv)
    res = run_bass_kernel_spmd(nc, in_maps, core_ids=list(range(8)), **spmd_kwargs)
    out = np.empty((B, S, D), dtype=np.float32)
    for c in range(8):
        b, h = c // 2, c % 2
        yc = res.results[c]["y"]
        for j in range(NSLOT):
            g = 2 * j + h
            out[b, g * P : (g + 1) * P, :] = yc[j * P : (j + 1) * P, :]
    return out, res


def kernel(x, Wq, Wk, Wv):
    out, _ = run(x, Wq, Wk, Wv)
    return out


# revision 23
# speedup vs baseline: 1.1477x; 1.0185x over previous
"""Causal single-head attention (B=4, S=2048, D=1024, f32) on 8 trn2 cores.

Sharding: data-parallel over batch (4) x 2-way split per batch, with the
K/V projections split by sequence parity and exchanged between the pair
via a 2-core AllGather (no duplicated projection work).

Core c handles batch b=c//2 and parity h=c%2: its query rows (and its
share of the K/V projection rows) are the 128-row blocks {2j+h : j=0..7}.
Each core projects Q/K/V only for its own 1024 rows, the pair exchanges
the K/V halves through DRAM bounce buffers + AllGather (replica groups
[[0,1],[2,3],[4,5],[6,7]]), and every core then runs block-causal
attention for its queries against the full 2048-row K/V.

K/V live in SBUF in *global* parity-block order (parity-0 blocks
ascending in the first half, parity-1 in the second) — exactly the
AllGather output order — so the instruction stream is identical on all
8 cores.  The residual per-core causal differences (whether the
boundary block of each parity is diagonal / fully-valid / fully-masked)
are carried by two 128x128 multiplicative mask inputs mA/mB.

All matmuls run in bf16 (inputs pre-cast/pre-transposed on the host),
f32 PSUM accumulation, f32 softmax normalization and f32 output.
Scores are computed transposed (s^T[k,q]) per key block with the widest
legal free dim, exp(scale*s) goes psum->sbuf on the scalar engine with
no max pass (|scale*s| <= ~5 for this data), p^T is the stationary
operand of the context matmul, and row sums come from ones^T @ p^T
matmuls accumulated in PSUM.

A tiny warmup AllGather issues at kernel start so the NRT collective
channel setup (~40us) overlaps with the projection compute instead of
delaying the K exchange.
"""

import numpy as np
import ml_dtypes

B, S, D = 4, 2048, 1024
P = 128
DI = D // P          # 8 contraction subtiles
NBLK = S // P        # 16 sequence blocks
NSLOT = 8            # query blocks per core
QCORE = NSLOT * P    # 1024 query rows per core
HALF = S // 2        # 1024 own-parity rows per core
SCALE = 1.0 / 32.0   # 1/sqrt(D)
BF16 = ml_dtypes.bfloat16
GROUPS = [[0, 1], [2, 3], [4, 5], [6, 7]]

_PROGRAM = None


def _build_program():
    import concourse.bacc as bacc
    import concourse.mybir as mybir
    import concourse.tile as tile

    dt = mybir.dt
    f32 = dt.float32
    bf = dt.bfloat16
    Exp = mybir.ActivationFunctionType.Exp

    nc = bacc.Bacc("TRN2")
    xT = nc.dram_tensor("xT", [D, HALF], bf, kind="ExternalInput")
    wqT = nc.dram_tensor("wqT", [D, D], bf, kind="ExternalInput")
    wkT = nc.dram_tensor("wkT", [D, D], bf, kind="ExternalInput")
    wvT = nc.dram_tensor("wvT", [D, D], bf, kind="ExternalInput")
    mA = nc.dram_tensor("mA", [P, P], bf, kind="ExternalInput")
    mB = nc.dram_tensor("mB", [P, P], bf, kind="ExternalInput")
    y = nc.dram_tensor("y", [QCORE, D], f32, kind="ExternalOutput")

    with (
        tile.TileContext(nc) as tc,
        tc.tile_pool(name="pers", bufs=1) as pers,
        tc.tile_pool(name="dram", bufs=1, space="DRAM") as dram,
    ):
        # warmup collective: pays the NRT channel-setup latency up front.
        # Its input is a tiny dram->dram copy issued before the big input
        # loads so its completion (and the CC doorbell) isn't queued behind
        # 8MB of weight traffic.
        warm_in = dram.tile([1, P], bf, tag="warm_in", name="warm_in")
        warm_out = dram.tile([2, P], bf, tag="warm_out", name="warm_out")
        nc.gpsimd.dma_start(warm_in[:], mA[0:1, :])
        nc.gpsimd.collective_compute(
            "AllGather",
            mybir.AluOpType.bypass,
            replica_groups=GROUPS,
            ins=[warm_in.opt()],
            outs=[warm_out.opt()],
        )

        mA_sb = pers.tile([P, P], bf, tag="mA", name="mA")
        nc.gpsimd.dma_start(mA_sb[:], mA[:])
        mB_sb = pers.tile([P, P], bf, tag="mB", name="mB")
        nc.gpsimd.dma_start(mB_sb[:], mB[:])
        ones_sb = pers.tile([P, 1], bf, tag="ones", name="ones")
        nc.gpsimd.memset(ones_sb[:], 1.0)

        # persistent per-core tensors (bf16), global parity-block layout:
        # kT [o, s_local], qT [o, q], v [s_local, o]
        kT_o = [pers.tile([P, S], bf, tag=f"kT{oi}", name=f"kT{oi}") for oi in range(DI)]
        qT_o = [pers.tile([P, QCORE], bf, tag=f"qT{oi}", name=f"qT{oi}") for oi in range(DI)]
        v_s = [pers.tile([P, D], bf, tag=f"v{si}", name=f"v{si}") for si in range(NBLK)]

        # bounce buffers for the K/V halves exchange (Shared outputs are
        # only supported for >4-core groups, so these stay Local).  V is
        # exchanged in two o-column halves so the first context matmuls
        # (o-cols 0:512) unblock before the whole exchange finishes.
        kb_in = dram.tile([HALF, HALF], bf, tag="kb_in", name="kb_in")
        kb_out = dram.tile([S, HALF], bf, tag="kb_out", name="kb_out")
        vb_in = [
            dram.tile([HALF, 512], bf, tag=f"vb_in{c}", name=f"vb_in{c}")
            for c in range(2)
        ]
        vb_out = [
            dram.tile([S, 512], bf, tag=f"vb_out{c}", name=f"vb_out{c}")
            for c in range(2)
        ]

        # ---- load + projections (own 1024 rows only) ----
        NSC = HALF // 512  # 2 s-chunks
        with (
            tc.tile_pool(name="ld", bufs=1) as ld,
            tc.tile_pool(name="ppsum", bufs=4, space="PSUM") as ppsum,
        ):
            xT3 = xT.rearrange("(di p) s -> di p s", p=P)
            wq3 = wqT.rearrange("(di p) o -> di p o", p=P)
            wk3 = wkT.rearrange("(di p) o -> di p o", p=P)
            wv3 = wvT.rearrange("(di p) o -> di p o", p=P)
            # Inputs spread across the three DMA-capable queues, ordered by
            # first use so the first projection groups' operands land first:
            # sync: wk then wq; scalar: xT first halves then second halves;
            # gpsimd: wv (after the tiny warmup-CC traffic).
            wk_d, xT_t, wv_d, wq_d = [], [], [], []
            for di in range(DI):
                t = ld.tile([P, D], bf, tag=f"wk{di}", name=f"wk{di}")
                nc.sync.dma_start(t[:], wk3[di])
                wk_d.append(t)
            for di in range(DI):
                t = ld.tile([P, HALF], bf, tag=f"xT{di}", name=f"xT{di}")
                nc.scalar.dma_start(t[:, 0:512], xT3[di, :, 0:512])
                xT_t.append(t)
            for di in range(DI):
                t = ld.tile([P, D], bf, tag=f"wv{di}", name=f"wv{di}")
                nc.gpsimd.dma_start(t[:], wv3[di])
                wv_d.append(t)
            for di in range(DI):
                nc.scalar.dma_start(xT_t[di][:, 512:HALF], xT3[di, :, 512:HALF])
            for di in range(DI):
                t = ld.tile([P, D], bf, tag=f"wq{di}", name=f"wq{di}")
                nc.sync.dma_start(t[:], wq3[di])
                wq_d.append(t)

            def proj(lhs_fn, rhs_fn, out_tile, out_cols, n):
                ps = ppsum.tile([P, n], f32, tag="ppsum", name="ppsum")
                for di in range(DI):
                    nc.tensor.matmul(
                        ps[:],
                        lhs_fn(di),
                        rhs_fn(di),
                        start=(di == 0),
                        stop=(di == DI - 1),
                    )
                nc.vector.tensor_copy(out_tile[:, out_cols], ps[:])

            with tc.tile_pool(name="kst", bufs=1) as kst:
                kstage = [
                    kst.tile([P, HALF], bf, tag=f"ks{oi}", name=f"ks{oi}")
                    for oi in range(DI)
                ]
                for sc in range(NSC):  # kT own half: [o, s_own]
                    cols = slice(sc * 512, (sc + 1) * 512)
                    for oi in range(DI):
                        oc = slice(oi * P, (oi + 1) * P)
                        proj(
                            lambda di, oc=oc: wk_d[di][:, oc],
                            lambda di, cols=cols: xT_t[di][:, cols],
                            kstage[oi], cols, 512,
                        )
                for oi in range(DI):  # stage out + exchange K
                    nc.gpsimd.dma_start(
                        kb_in[oi * P : (oi + 1) * P, :], kstage[oi][:]
                    )
            nc.gpsimd.collective_compute(
                "AllGather",
                mybir.AluOpType.bypass,
                replica_groups=GROUPS,
                ins=[kb_in.opt()],
                outs=[kb_out.opt()],
            )

            with tc.tile_pool(name="vst", bufs=1) as vst:
                vstage = [
                    vst.tile([P, D], bf, tag=f"vs{si}", name=f"vs{si}")
                    for si in range(NSLOT)
                ]
                for si in range(NSLOT):  # v own half: [s_own, o]
                    lo = si * P
                    for oh in range(D // 512):
                        cols = slice(oh * 512, (oh + 1) * 512)
                        proj(
                            lambda di, lo=lo: xT_t[di][:, lo : lo + P],
                            lambda di, cols=cols: wv_d[di][:, cols],
                            vstage[si], cols, 512,
                        )
                for si in range(NSLOT):
                    nc.gpsimd.dma_start(
                        vb_in[0][si * P : (si + 1) * P, :], vstage[si][:, 0:512]
                    )
                nc.gpsimd.collective_compute(
                    "AllGather",
                    mybir.AluOpType.bypass,
                    replica_groups=GROUPS,
                    ins=[vb_in[0].opt()],
                    outs=[vb_out[0].opt()],
                )
                for si in range(NSLOT):
                    nc.gpsimd.dma_start(
                        vb_in[1][si * P : (si + 1) * P, :], vstage[si][:, 512:D]
                    )
            nc.gpsimd.collective_compute(
                "AllGather",
                mybir.AluOpType.bypass,
                replica_groups=GROUPS,
                ins=[vb_in[1].opt()],
                outs=[vb_out[1].opt()],
            )

            for qc in range(QCORE // 512):  # qT: [o, q] own rows
                cols = slice(qc * 512, (qc + 1) * 512)
                for oi in range(DI):
                    oc = slice(oi * P, (oi + 1) * P)
                    proj(
                        lambda di, oc=oc: wq_d[di][:, oc],
                        lambda di, cols=cols: xT_t[di][:, cols],
                        qT_o[oi], cols, 512,
                    )

            # load back the gathered K/V, split across both free rings and
            # ordered by first consumption: K parity-0 (first score blocks)
            # on scalar, K parity-1 on sync; V o-cols 0:512 on scalar in
            # slot order (v0,v8,v1,v9,...), V o-cols 512: on sync.
            for oi in range(DI):
                nc.scalar.dma_start(
                    kT_o[oi][:, 0:HALF], kb_out[oi * P : (oi + 1) * P, :]
                )
                nc.sync.dma_start(
                    kT_o[oi][:, HALF:S],
                    kb_out[HALF + oi * P : HALF + (oi + 1) * P, :],
                )
            for si in range(NSLOT):
                nc.scalar.dma_start(
                    v_s[si][:, 0:512], vb_out[0][si * P : (si + 1) * P, :]
                )
                nc.scalar.dma_start(
                    v_s[NSLOT + si][:, 0:512],
                    vb_out[0][HALF + si * P : HALF + (si + 1) * P, :],
                )
            for si in range(NSLOT):
                nc.sync.dma_start(
                    v_s[si][:, 512:D], vb_out[1][si * P : (si + 1) * P, :]
                )
                nc.sync.dma_start(
                    v_s[NSLOT + si][:, 512:D],
                    vb_out[1][HALF + si * P : HALF + (si + 1) * P, :],
                )

        # ---- attention ----
        # local key block kb: 0..7 -> global 2*kb (parity 0),
        #                     8..15 -> global 2*(kb-8)+1 (parity 1).
        # key block kb is attended by query slots j >= kb%8.
        pT = [pers.tile([P, QCORE], bf, tag=f"pT{kb}", name=f"pT{kb}") for kb in range(NBLK)]
        with (
            tc.tile_pool(name="spsum", bufs=2, space="PSUM") as spsum,
            tc.tile_pool(name="rpsum", bufs=2, space="PSUM") as rpsum,
            tc.tile_pool(name="cpsum", bufs=4, space="PSUM") as cpsum,
            tc.tile_pool(name="small", bufs=2) as small,
            tc.tile_pool(name="reciprocals", bufs=1) as rpool,
        ):
            # scores s^T[k, q] per key block, widest legal free dim
            for kb in range(NBLK):
                ksl = slice(kb * P, (kb + 1) * P)
                q0 = (kb % NSLOT) * P
                while q0 < QCORE:
                    w = min(512, QCORE - q0)
                    ps = spsum.tile([P, 512], f32, tag="spsum", name="spsum")
                    for oi in range(DI):
                        nc.tensor.matmul(
                            ps[:, 0:w],
                            kT_o[oi][:, ksl],
                            qT_o[oi][:, q0 : q0 + w],
                            start=(oi == 0),
                            stop=(oi == DI - 1),
                        )
                    nc.scalar.activation(
                        pT[kb][:, q0 : q0 + w], ps[:, 0:w], Exp, scale=SCALE
                    )
                    q0 += w
            # boundary masks (multiplicative, post-exp)
            for j in range(NSLOT):
                qc = slice(j * P, (j + 1) * P)
                nc.vector.tensor_mul(pT[j][:, qc], pT[j][:, qc], mA_sb[:])
                nc.vector.tensor_mul(
                    pT[NSLOT + j][:, qc], pT[NSLOT + j][:, qc], mB_sb[:]
                )
            # row sums (pT.T @ ones -> [128,1] psum, q on partitions)
            # and context: ctx[q, o] = sum_k p^T[k,q] * v[k,o]
            for j in range(NSLOT):  # ascending: small slots pipeline with
                qc = slice(j * P, (j + 1) * P)  # the arriving V tiles
                kbs = list(range(0, j + 1)) + list(range(NSLOT, NSLOT + j + 1))
                rsp = rpsum.tile([P, 1], f32, tag="rsp", name="rsp")
                for i, kb in enumerate(kbs):
                    nc.tensor.matmul(
                        rsp[:],
                        pT[kb][:, qc],
                        ones_sb[:, 0:1],
                        start=(i == 0),
                        stop=(i == len(kbs) - 1),
                    )
                recip = rpool.tile([P, 1], f32, tag=f"recip{j}", name=f"recip{j}")
                nc.vector.reciprocal(recip[:], rsp[:])
                for oh in range(D // 512):
                    ocols = slice(oh * 512, (oh + 1) * 512)
                    cps = cpsum.tile([P, 512], f32, tag="cpsum", name="cpsum")
                    for i, kb in enumerate(kbs):
                        nc.tensor.matmul(
                            cps[:],
                            pT[kb][:, qc],
                            v_s[kb][:, ocols],
                            start=(i == 0),
                            stop=(i == len(kbs) - 1),
                        )
                    ct = small.tile([P, 512], f32, tag="ct", name="ct")
                    nc.vector.tensor_scalar_mul(ct[:], cps[:], recip[:, 0:1])
                    nc.sync.dma_start(y[qc, ocols], ct[:])
    nc.finalize()
    return nc


def _get_program():
    global _PROGRAM
    if _PROGRAM is None:
        _PROGRAM = _build_program()
    return _PROGRAM


def _host_prep(x, Wq, Wk, Wv):
    """Per-core input maps: transposed/cast weights, own-parity x^T, masks."""
    x = np.asarray(x, dtype=np.float32)
    tri_np = (np.arange(P)[None, :] >= np.arange(P)[:, None]).astype(BF16)
    ones_np = np.ones((P, P), dtype=BF16)
    zeros_np = np.zeros((P, P), dtype=BF16)
    wqT = np.ascontiguousarray(np.asarray(Wq, dtype=np.float32).T).astype(BF16)
    wkT = np.ascontiguousarray(np.asarray(Wk, dtype=np.float32).T).astype(BF16)
    wvT = np.ascontiguousarray(np.asarray(Wv, dtype=np.float32).T).astype(BF16)
    in_maps = []
    for c in range(8):
        b, h = c // 2, c % 2
        own = [2 * j + h for j in range(NSLOT)]
        xTb = np.asarray(x[b]).T.reshape(D, NBLK, P)[:, own, :].reshape(D, HALF)
        in_maps.append(
            {
                "xT": np.ascontiguousarray(xTb).astype(BF16),
                "wqT": wqT,
                "wkT": wkT,
                "wvT": wvT,
                "mA": tri_np if h == 0 else ones_np,
                "mB": zeros_np if h == 0 else tri_np,
            }
        )
    return in_maps


def run(x, Wq, Wk, Wv, **spmd_kwargs):
    """Run on all 8 cores; returns (out [B,S,D] f32, BassKernelResults)."""
    from concourse.bass_utils import run_bass_kernel_spmd

    nc = _get_program()
    in_maps = _host_prep(x, Wq, Wk, Wv)
    res = run_bass_kernel_spmd(nc, in_maps, core_ids=list(range(8)), **spmd_kwargs)
    out = np.empty((B, S, D), dtype=np.float32)
    for c in range(8):
        b, h = c // 2, c % 2
        yc = res.results[c]["y"]
        for j in range(NSLOT):
            g = 2 * j + h
            out[b, g * P : (g + 1) * P, :] = yc[j * P : (j + 1) * P, :]
    return out, res


def kernel(x, Wq, Wk, Wv):
    out, _ = run(x, Wq, Wk, Wv)
    return out
